# revision 11
# baseline (speedup 1.0000x reference)
"""Trainium2 Bass kernel for nn_DenseEmbed: out[t,b,i,e] = x[t,b,i] * W[i,e] + b[e].

Shapes (hardcoded): x (8, 64, 512) f32, W (512, 256) f32, b (256,) f32.
Output: (8, 64, 512, 256) f32 = 256 MiB.

Strategy: data-parallel over the leading T axis (8 values -> 8 NeuronCores).
Per core: out_c[n, i, e] = x_c[n, i] * W[i, e] (+ b[e]) with n in [0,64),
i in [0,512), e in [0,256).

Device dataflow per core:
  - W resident in SBUF as (128, 4*256): partition p, free (k, e), i = k*128+p.
  - x resident in SBUF as (128, 4*64): partition p, free (k, n).
  - For each n-block of NB and each k-tile: NB DVE tensor_scalar_mul ops
    (per-partition scalar = x[:, k, n]) fill a (128, NB*256) SBUF tile,
    which is stored to HBM with one HWDGE DMA.

The problem is HBM-write-bound (~32 MiB out per core); DVE compute (~50 us)
hides under the DMA stream. The device output tensor is written i-major
(D, N, E) so each DMA descriptor covers NB*1KiB contiguous HBM per
partition (4-KiB packets instead of 1-KiB); the host undoes the (n, i)
swap with a cheap strided copy during final assembly.
"""

import numpy as np

T, B, D, E = 8, 64, 512, 256
N_CORES = 8
KT = D // 128          # 4 k-tiles (partition blocks of i)
NB = 8                 # n-values per output tile (1 MiB tiles)
NB_PRO = 2             # smaller prologue tiles: output stream starts earlier
N_PER_CORE = T * B // N_CORES  # 64

I_MAJOR = True         # device out layout (D, N, E) vs (N, D, E)

_compiled = {}


def _build(with_bias: bool, i_major: bool):
    import concourse.tile as tile
    from concourse import bacc, mybir

    f32 = mybir.dt.float32
    nc = bacc.Bacc(
        "TRN2",
        target_bir_lowering=False,
        debug=False,
        num_devices=N_CORES,
    )
    x_d = nc.dram_tensor("x", [128, KT * N_PER_CORE], f32, kind="ExternalInput")
    w_d = nc.dram_tensor("w", [128, KT * E], f32, kind="ExternalInput")
    if with_bias:
        b_d = nc.dram_tensor("b", [128, E], f32, kind="ExternalInput")
    out_shape = [D, N_PER_CORE, E] if i_major else [N_PER_CORE, D, E]
    out_d = nc.dram_tensor("out", out_shape, f32, kind="ExternalOutput")

    with tile.TileContext(nc) as tc:
        with (
            tc.tile_pool(name="consts", bufs=1) as cpool,
            tc.tile_pool(name="outs", bufs=8) as opool,
        ):
            # Resident loads: two whole-tensor DMAs (big descriptors), queued
            # on the sync ring ahead of the output stream.
            w_sb = cpool.tile([128, KT * E], f32)
            x_sb = cpool.tile([128, KT * N_PER_CORE], f32)
            nc.sync.dma_start(out=x_sb[:], in_=x_d[:])
            nc.sync.dma_start(out=w_sb[:], in_=w_d[:])
            if with_bias:
                b_sb = cpool.tile([128, E], f32)
                nc.sync.dma_start(out=b_sb[:], in_=b_d[:])

            # n-blocks: small prologue tiles first so the output DMA stream
            # starts as early as possible, then steady-state NB-sized tiles.
            blocks = [NB_PRO, NB - NB_PRO] if NB_PRO else [NB]
            blocks += [NB] * ((N_PER_CORE - sum(blocks)) // NB)
            assert sum(blocks) == N_PER_CORE

            dma_engines = [nc.sync]
            dma_i = 0
            n0 = 0
            for blk in blocks:
                for k in range(KT):
                    t = opool.tile([128, blk * E], f32, tag="outs")
                    for j in range(blk):
                        n = n0 + j
                        dst = t[:, j * E:(j + 1) * E]
                        nc.vector.tensor_scalar_mul(
                            dst,
                            w_sb[:, k * E:(k + 1) * E],
                            x_sb[:, k * N_PER_CORE + n:k * N_PER_CORE + n + 1],
                        )
                        if with_bias:
                            nc.vector.tensor_add(dst, dst, b_sb[:])
                    if i_major:
                        dest = out_d[k * 128:(k + 1) * 128, n0:n0 + blk, :]
                    else:
                        dest = out_d[
                            n0:n0 + blk, k * 128:(k + 1) * 128, :
                        ].rearrange("n i e -> i n e")
                    dma_engines[dma_i % len(dma_engines)].dma_start(
                        out=dest,
                        in_=t[:].rearrange("p (n e) -> p n e", n=blk),
                    )
                    dma_i += 1
                n0 += blk
    nc.compile()
    return nc


def _get_nc(with_bias: bool, i_major: bool = I_MAJOR):
    key = (with_bias, i_major)
    if key not in _compiled:
        _compiled[key] = _build(with_bias, i_major)
    return _compiled[key]


def _pack_x_core(xc: np.ndarray) -> np.ndarray:
    # xc (64, 512) -> (128, 4*64): pk[p, k*64+n] = xc[n, k*128+p]
    return np.ascontiguousarray(
        xc.T.reshape(KT, 128, N_PER_CORE).transpose(1, 0, 2).reshape(128, -1)
    )


def _pack_w(W: np.ndarray) -> np.ndarray:
    # W (512, 256) -> (128, 4*256): pk[p, k*256+e] = W[k*128+p, e]
    return np.ascontiguousarray(
        W.reshape(KT, 128, E).transpose(1, 0, 2).reshape(128, -1)
    )


def _regen_missing():
    # setup_inputs() counterpart, in case W/b are not passed by the caller.
    import jax

    key = jax.random.key(0)
    _, kw = jax.random.split(key)
    limit = np.sqrt(6.0 / (D + E)).astype(np.float32)
    W = np.asarray(
        jax.random.uniform(
            kw, (D, E), dtype=np.float32, minval=-limit, maxval=limit
        )
    )
    b = np.zeros((E,), np.float32)
    return W, b


def _make_in_maps(x, W, b, with_bias):
    w_pk = _pack_w(W)
    x2 = x.reshape(N_CORES, N_PER_CORE, D)  # T-shard: core c <- t=c
    in_maps = []
    for c in range(N_CORES):
        m = {"x": _pack_x_core(x2[c]), "w": w_pk}
        if with_bias:
            m["b"] = np.ascontiguousarray(np.broadcast_to(b, (128, E)))
        in_maps.append(m)
    return in_maps


def _assemble(core_outs, i_major: bool = I_MAJOR):
    out = np.stack(core_outs, axis=0)
    if i_major:
        # (T, D, N, E) -> (T, N, D, E)
        out = np.ascontiguousarray(out.transpose(0, 2, 1, 3))
    return out.reshape(T, B, D, E)


def kernel(x=None, W=None, b=None, **_ignored):
    from concourse.bass_utils import run_bass_kernel_spmd

    x = np.ascontiguousarray(np.asarray(x, dtype=np.float32))
    assert x.shape == (T, B, D), x.shape
    if W is None or b is None:
        W_r, b_r = _regen_missing()
        W = W_r if W is None else W
        b = b_r if b is None else b
    W = np.ascontiguousarray(np.asarray(W, dtype=np.float32))
    b = np.ascontiguousarray(np.asarray(b, dtype=np.float32))

    with_bias = bool(np.any(b != 0.0))
    nc = _get_nc(with_bias)
    in_maps = _make_in_maps(x, W, b, with_bias)
    res = run_bass_kernel_spmd(nc, in_maps, list(range(N_CORES)))
    return _assemble([res.results[c]["out"] for c in range(N_CORES)])


# revision 15
# speedup vs baseline: 1.1804x; 1.1804x over previous
"""Trainium2 Bass kernel for nn_DenseEmbed: out[t,b,i,e] = x[t,b,i] * W[i,e] + b[e].

Shapes (hardcoded): x (8, 64, 512) f32, W (512, 256) f32, b (256,) f32.
Output: (8, 64, 512, 256) f32 = 256 MiB.

Strategy: data-parallel over the leading T axis (8 values -> 8 NeuronCores).
Per core: out_c[n, i, e] = x_c[n, i] * W[i, e] (+ b[e]) with n in [0,64),
i in [0,512), e in [0,256).

Device dataflow per core:
  - W resident in SBUF as (128, 4*256): partition p, free (k, e), i = k*128+p.
  - x resident in SBUF as (128, 4*64): partition p, free (k, n).
  - For each n-block of NB and each k-tile: NB DVE tensor_scalar_mul ops
    (per-partition scalar = x[:, k, n]) fill a (128, NB*256) SBUF tile,
    which is stored to HBM with one HWDGE DMA.

The problem is HBM-write-bound (~32 MiB out per core); DVE compute (~50 us)
hides under the DMA stream. The device output tensor is written i-major
(D, N, E) so each DMA descriptor covers NB*1KiB contiguous HBM per
partition (4-KiB packets instead of 1-KiB); the host undoes the (n, i)
swap with a cheap strided copy during final assembly.
"""

import numpy as np

T, B, D, E = 8, 64, 512, 256
N_CORES = 8
KT = D // 128          # 4 k-tiles (partition blocks of i)
NB = 16                # n-values per steady-state output tile (2 MiB tiles)
PRO_BLOCKS = [2, 6, 8]  # graduated prologue: output stream starts early
N_PER_CORE = T * B // N_CORES  # 64

I_MAJOR = True         # device out layout (D, N, E) vs (N, D, E)

_compiled = {}


def _build(with_bias: bool, i_major: bool):
    import concourse.tile as tile
    from concourse import bacc, mybir

    f32 = mybir.dt.float32
    nc = bacc.Bacc(
        "TRN2",
        target_bir_lowering=False,
        debug=False,
        num_devices=N_CORES,
    )
    x_d = nc.dram_tensor("x", [128, KT * N_PER_CORE], f32, kind="ExternalInput")
    w_d = nc.dram_tensor("w", [128, KT * E], f32, kind="ExternalInput")
    if with_bias:
        b_d = nc.dram_tensor("b", [128, E], f32, kind="ExternalInput")
    out_shape = [D, N_PER_CORE, E] if i_major else [N_PER_CORE, D, E]
    out_d = nc.dram_tensor("out", out_shape, f32, kind="ExternalOutput")

    with tile.TileContext(nc) as tc:
        with (
            tc.tile_pool(name="consts", bufs=1) as cpool,
            tc.tile_pool(name="outs", bufs=6) as opool,
        ):
            # Resident loads: x whole (small), W split per k-tile so the first
            # compute only waits on x + W[k0], not all of W.
            w_sb = cpool.tile([128, KT * E], f32)
            x_sb = cpool.tile([128, KT * N_PER_CORE], f32)
            nc.sync.dma_start(out=x_sb[:], in_=x_d[:])
            for k in range(KT):
                nc.sync.dma_start(
                    out=w_sb[:, k * E:(k + 1) * E],
                    in_=w_d[:, k * E:(k + 1) * E],
                )
            if with_bias:
                b_sb = cpool.tile([128, E], f32)
                nc.sync.dma_start(out=b_sb[:], in_=b_d[:])

            # n-blocks: small prologue tiles first so the output DMA stream
            # starts as early as possible, then steady-state NB-sized tiles.
            blocks = list(PRO_BLOCKS)
            blocks += [NB] * ((N_PER_CORE - sum(blocks)) // NB)
            assert sum(blocks) == N_PER_CORE, blocks

            dma_engines = [nc.sync]
            dma_i = 0
            n0 = 0
            for blk in blocks:
                for k in range(KT):
                    t = opool.tile([128, blk * E], f32, tag="outs")
                    for j in range(blk):
                        n = n0 + j
                        dst = t[:, j * E:(j + 1) * E]
                        nc.vector.tensor_scalar_mul(
                            dst,
                            w_sb[:, k * E:(k + 1) * E],
                            x_sb[:, k * N_PER_CORE + n:k * N_PER_CORE + n + 1],
                        )
                        if with_bias:
                            nc.vector.tensor_add(dst, dst, b_sb[:])
                    if i_major:
                        dest = out_d[k * 128:(k + 1) * 128, n0:n0 + blk, :]
                    else:
                        dest = out_d[
                            n0:n0 + blk, k * 128:(k + 1) * 128, :
                        ].rearrange("n i e -> i n e")
                    dma_engines[dma_i % len(dma_engines)].dma_start(
                        out=dest,
                        in_=t[:].rearrange("p (n e) -> p n e", n=blk),
                    )
                    dma_i += 1
                n0 += blk
    nc.compile()
    return nc


def _get_nc(with_bias: bool, i_major: bool = I_MAJOR):
    key = (with_bias, i_major)
    if key not in _compiled:
        _compiled[key] = _build(with_bias, i_major)
    return _compiled[key]


def _pack_x_core(xc: np.ndarray) -> np.ndarray:
    # xc (64, 512) -> (128, 4*64): pk[p, k*64+n] = xc[n, k*128+p]
    return np.ascontiguousarray(
        xc.T.reshape(KT, 128, N_PER_CORE).transpose(1, 0, 2).reshape(128, -1)
    )


def _pack_w(W: np.ndarray) -> np.ndarray:
    # W (512, 256) -> (128, 4*256): pk[p, k*256+e] = W[k*128+p, e]
    return np.ascontiguousarray(
        W.reshape(KT, 128, E).transpose(1, 0, 2).reshape(128, -1)
    )


def _regen_missing():
    # setup_inputs() counterpart, in case W/b are not passed by the caller.
    import jax

    key = jax.random.key(0)
    _, kw = jax.random.split(key)
    limit = np.sqrt(6.0 / (D + E)).astype(np.float32)
    W = np.asarray(
        jax.random.uniform(
            kw, (D, E), dtype=np.float32, minval=-limit, maxval=limit
        )
    )
    b = np.zeros((E,), np.float32)
    return W, b


def _make_in_maps(x, W, b, with_bias):
    w_pk = _pack_w(W)
    x2 = x.reshape(N_CORES, N_PER_CORE, D)  # T-shard: core c <- t=c
    in_maps = []
    for c in range(N_CORES):
        m = {"x": _pack_x_core(x2[c]), "w": w_pk}
        if with_bias:
            m["b"] = np.ascontiguousarray(np.broadcast_to(b, (128, E)))
        in_maps.append(m)
    return in_maps


def _assemble(core_outs, i_major: bool = I_MAJOR):
    out = np.stack(core_outs, axis=0)
    if i_major:
        # (T, D, N, E) -> (T, N, D, E)
        out = np.ascontiguousarray(out.transpose(0, 2, 1, 3))
    return out.reshape(T, B, D, E)


def kernel(x=None, W=None, b=None, **_ignored):
    from concourse.bass_utils import run_bass_kernel_spmd

    x = np.ascontiguousarray(np.asarray(x, dtype=np.float32))
    assert x.shape == (T, B, D), x.shape
    if W is None or b is None:
        W_r, b_r = _regen_missing()
        W = W_r if W is None else W
        b = b_r if b is None else b
    W = np.ascontiguousarray(np.asarray(W, dtype=np.float32))
    b = np.ascontiguousarray(np.asarray(b, dtype=np.float32))

    with_bias = bool(np.any(b != 0.0))
    nc = _get_nc(with_bias)
    in_maps = _make_in_maps(x, W, b, with_bias)
    res = run_bass_kernel_spmd(nc, in_maps, list(range(N_CORES)))
    return _assemble([res.results[c]["out"] for c in range(N_CORES)])
